# revision 37
# baseline (speedup 1.0000x reference)
"""Trainium2 Bass kernel for nn_CNNTeacherModel_14551349198856 (moe_routing).

Reference computation: for each row i of hidden_state [8192, 1024]:
    out[i] = W[group[i]] @ hidden[i] + b[group[i]]   if group[i] < 5
    out[i] = float(labels[i])  (broadcast over L)    if group[i] == 5

Strategy (MoE routing — compute only the selected head per row, 5x fewer
FLOPs than the reference's all-heads einsum):
  * Host: sort active rows (group<5) by group, deal them round-robin to 4
    batch shards so every shard has identical per-group row counts (pad to
    a 128 multiple per group with dummy rows).  The L=1024 output dim is
    split in 2.  Core (s, l) of the 4x2 grid computes its shard's rows for
    L-half l.
  * Device (per core): x and W live in SBUF, loaded with a few big DMAs
    in host-packed [128, cols] layouts (2-8KB lines; HWDGE issue costs
    ~0.6us each, so transfer count matters).  Bias is broadcast once to
    [128, 512] per group via K=1 ones-matmuls.  For each 128-row M-tile
    (statically known group): 8 accumulating matmuls over the contraction
    (H) into one PSUM bank, then a VectorE eviction that adds the bias,
    and a per-tile store on the scalar HWDGE queue.
  * Transport dtype is bf16 (x, W, bias, y) to halve HBM traffic — the
    kernel is HBM-bound (~275 GB/s/core).  PSUM accumulates in fp32.
    Error vs the fp32 reference is ~1.3e-2 absolute on logits of scale ~3,
    i.e. ~1.3e-5 of the output absmax (label rows dominate at 1023).
    Set MOE_FP32R=1 for the fp32r path (~5e-4 absolute) at 2x DMA bytes.
  * A warmup chain of matmuls lifts the PE HAM clock-gate to 2.4 GHz
    while the first loads stream.
  * Host: scatter device outputs back by the inverse permutation; fill
    group==5 rows from labels.
"""

import math
import os

import numpy as np

B, H, L, NH = 8192, 1024, 1024, 5
PB, PL = 4, 2          # batch shards x L shards = 8 cores
LS = L // PL           # 512 output columns per core
KT = H // 128          # 8 contraction tiles
N_CORES = PB * PL
N_WARMUP = int(os.environ.get("MOE_WARMUP", "18"))
XSPLIT = int(os.environ.get("MOE_XSPLIT", "1"))   # DMAs per x M-tile load
WSPLIT = int(os.environ.get("MOE_WSPLIT", "2"))   # DMAs per W group load

USE_FP32R = bool(int(os.environ.get("MOE_FP32R", "0")))
USE_FP8 = bool(int(os.environ.get("MOE_FP8", "0")))
W_SCALE = 16.0  # fp8 path: W,b pre-scaled by this, undone at eviction

# stash of the last BassKernelResults (so a test harness can read
# exec_time_ns when tracing is enabled via BASS_TRACE)
LAST_RESULTS = None


def _split_excess_waits(nc, mybir, cap=1):
    """Walrus in this toolchain rejects >cap embedded sync-waits per
    instruction ("Too many sync wait commands").  Hoist excess waits into
    fresh same-engine InstNoOps placed immediately before the instruction
    (sequencers execute waits in stream order, so semantics are identical)."""
    for f in nc.m.functions:
        for blk in f.blocks:
            insts = list(blk.instructions)
            new = []
            changed = False
            for inst in insts:
                try:
                    si = inst.sync_info
                except AttributeError:
                    si = None
                waits = list(si.on_wait) if si else []
                if len(waits) > cap:
                    changed = True
                    excess, keep = waits[:-cap], waits[-cap:]
                    for i in range(0, len(excess), cap):
                        new.append(
                            mybir.InstNoOp(
                                name=nc.get_next_instruction_name(),
                                sync_info=mybir.SyncInfo(
                                    on_wait=excess[i:i + cap], on_update=[]
                                ),
                                bass_nofuse=True,
                                engine=inst.engine,
                            )
                        )
                    inst.sync_info = mybir.SyncInfo(
                        on_wait=keep, on_update=list(si.on_update)
                    )
                new.append(inst)
            if changed:
                blk.instructions = new


def _build_program(n_seg):
    """Build the per-core Bass program.  n_seg[g] = rows (multiple of 128)
    this core computes for group g; R = sum(n_seg).

    DRAM layouts (host-packed):
      xp  [128, T*KT*128] xp[p, (t*KT+h)*128 + r] = x_row[t*128+r][h*128+p]
                          (tile-major so each M-tile is one contiguous load)
      wp  [128, NH*KT*LS] wp[p, (g*KT+h)*LS + j]  = W[g][l0+j, h*128+p]
      bp  [1, NH*LS]      bp[0, g*LS + j]         = b[g, l0+j]
      y   [128, T*LS]     y[p, t*LS + j] = out row (t*128+p) col j   (T tiles)
    """
    import concourse.bass as bass
    import concourse.mybir as mybir
    import concourse.tile as tile

    R = sum(n_seg)
    T = R // 128
    f32 = mybir.dt.float32
    if USE_FP32R:
        mm_dt, io_dt = mybir.dt.float32r, mybir.dt.float32
    elif USE_FP8:
        mm_dt, io_dt = mybir.dt.float8e4, mybir.dt.bfloat16
    else:
        mm_dt, io_dt = mybir.dt.bfloat16, mybir.dt.bfloat16

    nc = bass.Bass()
    xdr = nc.dram_tensor("xp", [128, KT * R], mm_dt, kind="ExternalInput")
    wdr = nc.dram_tensor("wp", [128, NH * KT * LS], mm_dt, kind="ExternalInput")
    bdr = nc.dram_tensor("bp", [1, NH * LS], mm_dt, kind="ExternalInput")
    y = nc.dram_tensor("y", [128, T * LS], io_dt, kind="ExternalOutput")

    with tile.TileContext(nc) as tc:
        with (
            tc.tile_pool(name="xp_sb", bufs=1) as xp_sb,
            tc.tile_pool(name="wp_sb", bufs=1) as wp_sb,
            tc.tile_pool(name="cp", bufs=1) as cp,
            tc.tile_pool(name="pp", bufs=5, space="PSUM") as pp,
            tc.tile_pool(name="wup", bufs=1, space="PSUM") as wup,
            tc.tile_pool(name="op", bufs=3) as op,
        ):
            # --- PE warmup: keep the HAM clock-gate open while DMAs stream.
            # The psum bank is never read.
            wu_x = cp.tile([128, 128], mm_dt, tag="wux", name="wux")
            wu_w = cp.tile([128, LS], mm_dt, tag="wuw", name="wuw")
            nc.gpsimd.memset(wu_x[:], 0.0)
            nc.gpsimd.memset(wu_w[:], 0.0)
            wu_ps = wup.tile([128, LS], f32, name="wups")
            for _ in range(N_WARMUP):
                nc.tensor.matmul(wu_ps[:], wu_x[:], wu_w[:], start=True, stop=True)

            # --- ones row; bias rows tile; broadcast bias to [128, LS] per
            # group once via K=1 matmuls (2 rotating banks), evictions then
            # add it on the VectorE instead of a per-tile PE matmul
            ones_t = cp.tile([1, 128], mm_dt, tag="ones", name="ones")
            nc.vector.memset(ones_t[:], 1.0)
            bias_t = cp.tile([1, NH * LS], mm_dt, tag="bias", name="bias")
            nc.scalar.dma_start(out=bias_t[:], in_=bdr[:])
            bias_bc = []
            bps = [wup.tile([128, LS], f32, name=f"bps{i}") for i in range(2)]
            for g in range(NH):
                bb_t = cp.tile([128, LS], f32, tag=f"bb{g}", name=f"bb{g}")
                nc.tensor.matmul(
                    bps[g % 2][:], ones_t[:], bias_t[0:1, g * LS:(g + 1) * LS],
                    start=True, stop=True,
                )
                if USE_FP8:
                    nc.vector.tensor_scalar_mul(
                        bb_t[:], bps[g % 2][:], 1.0 / W_SCALE
                    )
                else:
                    nc.vector.tensor_copy(bb_t[:], bps[g % 2][:])
                bias_bc.append(bb_t)

            # two HWDGE queues (SP + ACT); alternate the big loads
            ld_engines = [nc.sync, nc.scalar]

            # x loads: one contiguous DMA per M-tile (tile-major packing);
            # W loads: one DMA per group.  Issue in consumption order,
            # alternating the two HWDGE queues.
            TKT = KT * 128
            wts = []
            xtiles = []
            ld_i = 0
            tglob = 0
            first_seg = True
            for g in range(NH):
                ng = n_seg[g]
                if ng == 0:
                    wts.append(None)
                    continue
                # seg 0: split its x tile in 2 and W in 4 so both queues
                # carry the first tile's dependencies in parallel; then reset
                # the alternation counter so every later load keeps the exact
                # queue assignment of the tuned schedule
                xs = 2 if first_seg else XSPLIT
                ws = 4 if first_seg else WSPLIT
                xt_first = xp_sb.tile([128, TKT], mm_dt, tag=f"xt{tglob}",
                                      name=f"xt{tglob}")
                xc = TKT // xs
                for j in range(xs):
                    ld_engines[ld_i % 2].dma_start(
                        out=xt_first[:, j * xc:(j + 1) * xc],
                        in_=xdr[:, tglob * TKT + j * xc:tglob * TKT + (j + 1) * xc],
                    )
                    ld_i += 1
                xtiles.append(xt_first)
                tglob += 1
                wt_t = wp_sb.tile([128, KT * LS], mm_dt, tag=f"w{g}", name=f"w{g}")
                wc = KT * LS // ws
                for j in range(ws):
                    ld_engines[ld_i % 2].dma_start(
                        out=wt_t[:, j * wc:(j + 1) * wc],
                        in_=wdr[:, g * KT * LS + j * wc:g * KT * LS + (j + 1) * wc],
                    )
                    ld_i += 1
                wts.append(wt_t)
                if first_seg:
                    ld_i = 1 + WSPLIT  # as if x(1) + w(WSPLIT) had been issued
                    first_seg = False
                xc = TKT // XSPLIT
                for t in range(1, ng // 128):
                    xt_t = xp_sb.tile([128, TKT], mm_dt, tag=f"xt{tglob}",
                                      name=f"xt{tglob}")
                    for j in range(XSPLIT):
                        ld_engines[ld_i % 2].dma_start(
                            out=xt_t[:, j * xc:(j + 1) * xc],
                            in_=xdr[:, tglob * TKT + j * xc:tglob * TKT + (j + 1) * xc],
                        )
                        ld_i += 1
                    xtiles.append(xt_t)
                    tglob += 1

            tglob = 0
            for g in range(NH):
                ng = n_seg[g]
                if ng == 0:
                    continue
                nt = ng // 128
                ot = op.tile([128, nt * LS], io_dt, tag="ot", name=f"ot{g}")
                for t in range(nt):
                    ps = pp.tile([128, LS], f32, tag="ps", name=f"ps{g}_{t}")
                    xt_t = xtiles[tglob + t]
                    for h in range(KT):
                        nc.tensor.matmul(
                            ps[:],
                            xt_t[:, h * 128:(h + 1) * 128],
                            wts[g][:, h * LS:(h + 1) * LS],
                            start=(h == 0),
                            stop=(h == KT - 1),
                        )
                    if USE_FP8:
                        nc.vector.scalar_tensor_tensor(
                            ot[:, t * LS:(t + 1) * LS], ps[:], 1.0 / W_SCALE,
                            bias_bc[g][:], mybir.AluOpType.mult,
                            mybir.AluOpType.add,
                        )
                    else:
                        nc.vector.tensor_add(
                            ot[:, t * LS:(t + 1) * LS], ps[:], bias_bc[g][:]
                        )
                    # per-tile store on the scalar HWDGE queue (idle once
                    # loads finish) so the kernel tail is one small store
                    nc.scalar.dma_start(
                        out=y[:, (tglob + t) * LS:(tglob + t + 1) * LS],
                        in_=ot[:, t * LS:(t + 1) * LS],
                    )
                tglob += nt

    _split_excess_waits(nc, mybir)
    return nc


def _ensure_axon_hooks_importable():
    """bass_utils' BASS_TRACE path imports antenv.axon_hooks, which this
    image lacks; register a null shim so a stray BASS_TRACE env var can't
    crash the run (tracing then degrades to a logged skip)."""
    import sys
    import types

    try:
        import antenv.axon_hooks  # noqa: F401
    except ImportError:
        mod = types.ModuleType("antenv.axon_hooks")
        mod._hook = None
        mod.get_axon_ntff_profile_hook = lambda: getattr(
            sys.modules["antenv.axon_hooks"], "_hook", None
        )

        def _set(h):
            sys.modules["antenv.axon_hooks"]._hook = h

        mod.set_axon_ntff_profile_hook = _set
        sys.modules["antenv.axon_hooks"] = mod


def kernel(hidden_state, W, b, group, labels):
    global LAST_RESULTS
    import ml_dtypes
    _ensure_axon_hooks_importable()
    from concourse.bass_utils import run_bass_kernel_spmd

    hidden_state = np.ascontiguousarray(np.asarray(hidden_state, dtype=np.float32))
    W = np.asarray(W, dtype=np.float32)
    b = np.asarray(b, dtype=np.float32)
    group = np.asarray(group)
    labels = np.asarray(labels)

    if USE_FP32R:
        np_x = np_w = np_io = np.float32
        wscale = 1.0
    elif USE_FP8:
        np_x = np_w = ml_dtypes.float8_e4m3
        np_io = ml_dtypes.bfloat16
        wscale = W_SCALE
    else:
        np_x = np_w = np_io = ml_dtypes.bfloat16
        wscale = 1.0

    g64 = group.astype(np.int64)
    active = np.nonzero(g64 < NH)[0]
    order = np.argsort(g64[active], kind="stable")
    sidx = active[order]
    counts = np.bincount(g64[active], minlength=NH)

    # per-shard rows per group, padded to a multiple of 128
    n_seg = []
    for g in range(NH):
        n = math.ceil(counts[g] / PB) if counts[g] else 0
        n_seg.append(128 * math.ceil(n / 128) if n else 0)
    R = sum(n_seg)
    T = R // 128

    # deal rows: shard s takes every PB-th row of each group's sorted run
    idx = np.full((PB, R), -1, dtype=np.int64)
    off = 0
    roff = 0
    for g in range(NH):
        rows = sidx[off:off + counts[g]]
        for s in range(PB):
            sub = rows[s::PB]
            idx[s, roff:roff + len(sub)] = sub
        off += counts[g]
        roff += n_seg[g]

    # pack x per shard: [128, T*KT*128], M-tile-major so each tile is one
    # contiguous DMA: xp[p, (t*KT+h)*128 + r] = xg[t*128+r, h*128+p]
    xpacks = []
    for s in range(PB):
        xg = hidden_state[np.maximum(idx[s], 0)].astype(np_x)   # [R, H]
        xp = xg.reshape(T, 128, KT, 128).transpose(3, 0, 2, 1)  # [p, t, h, r]
        xpacks.append(np.ascontiguousarray(xp.reshape(128, T * KT * 128)))

    # pack W per L-half: [128, NH*KT*LS]; bias [1, NH*LS]
    wpacks = []
    bpacks = []
    for l in range(PL):
        parts = []
        for g in range(NH):
            wg = (W[g].T[:, l * LS:(l + 1) * LS] * wscale).astype(np_w)  # [H, LS]
            wg = wg.reshape(KT, 128, LS).transpose(1, 0, 2)     # [128, KT, LS]
            parts.append(wg.reshape(128, KT * LS))
        wpacks.append(np.ascontiguousarray(np.concatenate(parts, axis=1)))
        bpacks.append(
            np.ascontiguousarray(
                (b[:, l * LS:(l + 1) * LS] * wscale).astype(np_w).reshape(1, NH * LS)
            )
        )

    in_maps = []
    for c in range(N_CORES):
        s, l = divmod(c, PL)
        in_maps.append({"xp": xpacks[s], "wp": wpacks[l], "bp": bpacks[l]})

    nc = _build_program(n_seg)
    res = run_bass_kernel_spmd(nc, in_maps, list(range(N_CORES)))
    LAST_RESULTS = res

    out = np.empty((B, L), dtype=np.float32)
    lab_rows = g64 == NH
    out[lab_rows] = labels[lab_rows, None].astype(np.float32)
    for c in range(N_CORES):
        s, l = divmod(c, PL)
        yp = res.results[c]["y"].astype(np.float32)       # [128, T*LS]
        yg = yp.reshape(128, T, LS).transpose(1, 0, 2).reshape(R, LS)
        m = idx[s] >= 0
        out[idx[s][m], l * LS:(l + 1) * LS] = yg[m]
    return out


# revision 38
# speedup vs baseline: 1.1142x; 1.1142x over previous
"""Trainium2 Bass kernel for nn_CNNTeacherModel_14551349198856 (moe_routing).

Reference computation: for each row i of hidden_state [8192, 1024]:
    out[i] = W[group[i]] @ hidden[i] + b[group[i]]   if group[i] < 5
    out[i] = float(labels[i])  (broadcast over L)    if group[i] == 5

Strategy (MoE routing — compute only the selected head per row, 5x fewer
FLOPs than the reference's all-heads einsum):
  * Host: sort active rows (group<5) by group, deal them round-robin to 4
    batch shards so every shard has identical per-group row counts (pad to
    a 128 multiple per group with dummy rows).  The L=1024 output dim is
    split in 2.  Core (s, l) of the 4x2 grid computes its shard's rows for
    L-half l.
  * Device (per core): x and W live in SBUF, loaded with a few big DMAs
    in host-packed [128, cols] layouts (2-8KB lines; HWDGE issue costs
    ~0.6us each, so transfer count matters).  Bias is broadcast once to
    [128, 512] per group via K=1 ones-matmuls.  For each 128-row M-tile
    (statically known group): 8 accumulating matmuls over the contraction
    (H) into one PSUM bank, then a VectorE eviction that adds the bias,
    and a per-tile store on the scalar HWDGE queue.
  * Transport dtype is bf16 (x, W, bias, y) to halve HBM traffic — the
    kernel is HBM-bound (~275 GB/s/core).  PSUM accumulates in fp32.
    Error vs the fp32 reference is ~1.3e-2 absolute on logits of scale ~3,
    i.e. ~1.3e-5 of the output absmax (label rows dominate at 1023).
    Set MOE_FP32R=1 for the fp32r path (~5e-4 absolute) at 2x DMA bytes.
  * A warmup chain of matmuls lifts the PE HAM clock-gate to 2.4 GHz
    while the first loads stream.
  * Host: scatter device outputs back by the inverse permutation; fill
    group==5 rows from labels.
"""

import math
import os

import numpy as np

B, H, L, NH = 8192, 1024, 1024, 5
PB, PL = 4, 2          # batch shards x L shards = 8 cores
LS = L // PL           # 512 output columns per core
KT = H // 128          # 8 contraction tiles
N_CORES = PB * PL
N_WARMUP = int(os.environ.get("MOE_WARMUP", "18"))
XSPLIT = int(os.environ.get("MOE_XSPLIT", "1"))   # DMAs per x M-tile load
WSPLIT = int(os.environ.get("MOE_WSPLIT", "2"))   # DMAs per W group load

USE_FP32R = bool(int(os.environ.get("MOE_FP32R", "0")))
USE_FP8 = bool(int(os.environ.get("MOE_FP8", "0")))
W_SCALE = 16.0  # fp8 path: W,b pre-scaled by this, undone at eviction

# stash of the last BassKernelResults (so a test harness can read
# exec_time_ns when tracing is enabled via BASS_TRACE)
LAST_RESULTS = None


def _split_excess_waits(nc, mybir, cap=1):
    """Walrus in this toolchain rejects >cap embedded sync-waits per
    instruction ("Too many sync wait commands").  Hoist excess waits into
    fresh same-engine InstNoOps placed immediately before the instruction
    (sequencers execute waits in stream order, so semantics are identical)."""
    for f in nc.m.functions:
        for blk in f.blocks:
            insts = list(blk.instructions)
            new = []
            changed = False
            for inst in insts:
                try:
                    si = inst.sync_info
                except AttributeError:
                    si = None
                waits = list(si.on_wait) if si else []
                if len(waits) > cap:
                    changed = True
                    excess, keep = waits[:-cap], waits[-cap:]
                    for i in range(0, len(excess), cap):
                        new.append(
                            mybir.InstNoOp(
                                name=nc.get_next_instruction_name(),
                                sync_info=mybir.SyncInfo(
                                    on_wait=excess[i:i + cap], on_update=[]
                                ),
                                bass_nofuse=True,
                                engine=inst.engine,
                            )
                        )
                    inst.sync_info = mybir.SyncInfo(
                        on_wait=keep, on_update=list(si.on_update)
                    )
                new.append(inst)
            if changed:
                blk.instructions = new


def _build_program(n_seg):
    """Build the per-core Bass program.  n_seg[g] = rows (multiple of 128)
    this core computes for group g; R = sum(n_seg).

    DRAM layouts (host-packed):
      xp  [128, T*KT*128] xp[p, (t*KT+h)*128 + r] = x_row[t*128+r][h*128+p]
                          (tile-major so each M-tile is one contiguous load)
      wp  [128, NH*KT*LS] wp[p, (g*KT+h)*LS + j]  = W[g][l0+j, h*128+p]
      bp  [1, NH*LS]      bp[0, g*LS + j]         = b[g, l0+j]
      y   [128, T*LS]     y[p, t*LS + j] = out row (t*128+p) col j   (T tiles)
    """
    import concourse.bass as bass
    import concourse.mybir as mybir
    import concourse.tile as tile

    R = sum(n_seg)
    T = R // 128
    f32 = mybir.dt.float32
    if USE_FP32R:
        mm_dt, io_dt = mybir.dt.float32r, mybir.dt.float32
    elif USE_FP8:
        mm_dt, io_dt = mybir.dt.float8e4, mybir.dt.bfloat16
    else:
        mm_dt, io_dt = mybir.dt.bfloat16, mybir.dt.bfloat16

    nc = bass.Bass()
    xdr = nc.dram_tensor("xp", [128, KT * R], mm_dt, kind="ExternalInput")
    wdr = nc.dram_tensor("wp", [128, NH * KT * LS], mm_dt, kind="ExternalInput")
    bdr = nc.dram_tensor("bp", [1, NH * LS], mm_dt, kind="ExternalInput")
    y = nc.dram_tensor("y", [128, T * LS], io_dt, kind="ExternalOutput")

    with tile.TileContext(nc) as tc:
        with (
            tc.tile_pool(name="xp_sb", bufs=1) as xp_sb,
            tc.tile_pool(name="wp_sb", bufs=1) as wp_sb,
            tc.tile_pool(name="cp", bufs=1) as cp,
            tc.tile_pool(name="pp", bufs=5, space="PSUM") as pp,
            tc.tile_pool(name="wup", bufs=1, space="PSUM") as wup,
            tc.tile_pool(name="op", bufs=3) as op,
        ):
            # --- PE warmup: keep the HAM clock-gate open while DMAs stream.
            # The psum bank is never read.
            wu_x = cp.tile([128, 128], mm_dt, tag="wux", name="wux")
            wu_w = cp.tile([128, LS], mm_dt, tag="wuw", name="wuw")
            nc.gpsimd.memset(wu_x[:], 0.0)
            nc.gpsimd.memset(wu_w[:], 0.0)
            wu_ps = wup.tile([128, LS], f32, name="wups")
            for _ in range(N_WARMUP):
                nc.tensor.matmul(wu_ps[:], wu_x[:], wu_w[:], start=True, stop=True)

            # --- ones row; bias rows tile; broadcast bias to [128, LS] per
            # group once via K=1 matmuls (2 rotating banks), evictions then
            # add it on the VectorE instead of a per-tile PE matmul
            ones_t = cp.tile([1, 128], mm_dt, tag="ones", name="ones")
            nc.vector.memset(ones_t[:], 1.0)
            bias_t = cp.tile([1, NH * LS], mm_dt, tag="bias", name="bias")
            nc.scalar.dma_start(out=bias_t[:], in_=bdr[:])
            bias_bc = []
            bps = [wup.tile([128, LS], f32, name=f"bps{i}") for i in range(2)]
            for g in range(NH):
                bb_t = cp.tile([128, LS], f32, tag=f"bb{g}", name=f"bb{g}")
                nc.tensor.matmul(
                    bps[g % 2][:], ones_t[:], bias_t[0:1, g * LS:(g + 1) * LS],
                    start=True, stop=True,
                )
                if USE_FP8:
                    nc.vector.tensor_scalar_mul(
                        bb_t[:], bps[g % 2][:], 1.0 / W_SCALE
                    )
                else:
                    nc.vector.tensor_copy(bb_t[:], bps[g % 2][:])
                bias_bc.append(bb_t)

            # two HWDGE queues (SP + ACT); alternate the big loads
            ld_engines = [nc.sync, nc.scalar]

            # x loads: one contiguous DMA per M-tile (tile-major packing);
            # W loads: one DMA per group.  Issue in consumption order,
            # alternating the two HWDGE queues.
            TKT = KT * 128
            wts = []
            xtiles = []
            ld_i = 0
            tglob = 0
            for g in range(NH):
                ng = n_seg[g]
                if ng == 0:
                    wts.append(None)
                    continue
                xt_first = xp_sb.tile([128, TKT], mm_dt, tag=f"xt{tglob}",
                                      name=f"xt{tglob}")
                xc = TKT // XSPLIT
                for j in range(XSPLIT):
                    ld_engines[ld_i % 2].dma_start(
                        out=xt_first[:, j * xc:(j + 1) * xc],
                        in_=xdr[:, tglob * TKT + j * xc:tglob * TKT + (j + 1) * xc],
                    )
                    ld_i += 1
                xtiles.append(xt_first)
                tglob += 1
                wt_t = wp_sb.tile([128, KT * LS], mm_dt, tag=f"w{g}", name=f"w{g}")
                wc = KT * LS // WSPLIT
                for j in range(WSPLIT):
                    ld_engines[ld_i % 2].dma_start(
                        out=wt_t[:, j * wc:(j + 1) * wc],
                        in_=wdr[:, g * KT * LS + j * wc:g * KT * LS + (j + 1) * wc],
                    )
                    ld_i += 1
                wts.append(wt_t)
                for t in range(1, ng // 128):
                    xt_t = xp_sb.tile([128, TKT], mm_dt, tag=f"xt{tglob}",
                                      name=f"xt{tglob}")
                    for j in range(XSPLIT):
                        ld_engines[ld_i % 2].dma_start(
                            out=xt_t[:, j * xc:(j + 1) * xc],
                            in_=xdr[:, tglob * TKT + j * xc:tglob * TKT + (j + 1) * xc],
                        )
                        ld_i += 1
                    xtiles.append(xt_t)
                    tglob += 1

            tglob = 0
            for g in range(NH):
                ng = n_seg[g]
                if ng == 0:
                    continue
                nt = ng // 128
                ot = op.tile([128, nt * LS], io_dt, tag="ot", name=f"ot{g}")
                for t in range(nt):
                    ps = pp.tile([128, LS], f32, tag="ps", name=f"ps{g}_{t}")
                    xt_t = xtiles[tglob + t]
                    for h in range(KT):
                        nc.tensor.matmul(
                            ps[:],
                            xt_t[:, h * 128:(h + 1) * 128],
                            wts[g][:, h * LS:(h + 1) * LS],
                            start=(h == 0),
                            stop=(h == KT - 1),
                        )
                    if USE_FP8:
                        nc.vector.scalar_tensor_tensor(
                            ot[:, t * LS:(t + 1) * LS], ps[:], 1.0 / W_SCALE,
                            bias_bc[g][:], mybir.AluOpType.mult,
                            mybir.AluOpType.add,
                        )
                    else:
                        nc.vector.tensor_add(
                            ot[:, t * LS:(t + 1) * LS], ps[:], bias_bc[g][:]
                        )
                    # per-tile store on the scalar HWDGE queue (idle once
                    # loads finish) so the kernel tail is one small store
                    nc.scalar.dma_start(
                        out=y[:, (tglob + t) * LS:(tglob + t + 1) * LS],
                        in_=ot[:, t * LS:(t + 1) * LS],
                    )
                tglob += nt

    _split_excess_waits(nc, mybir)
    return nc


def _ensure_axon_hooks_importable():
    """bass_utils' BASS_TRACE path imports antenv.axon_hooks, which this
    image lacks; register a null shim so a stray BASS_TRACE env var can't
    crash the run (tracing then degrades to a logged skip)."""
    import sys
    import types

    try:
        import antenv.axon_hooks  # noqa: F401
    except ImportError:
        mod = types.ModuleType("antenv.axon_hooks")
        mod._hook = None
        mod.get_axon_ntff_profile_hook = lambda: getattr(
            sys.modules["antenv.axon_hooks"], "_hook", None
        )

        def _set(h):
            sys.modules["antenv.axon_hooks"]._hook = h

        mod.set_axon_ntff_profile_hook = _set
        sys.modules["antenv.axon_hooks"] = mod


def kernel(hidden_state, W, b, group, labels):
    global LAST_RESULTS
    import ml_dtypes
    _ensure_axon_hooks_importable()
    from concourse.bass_utils import run_bass_kernel_spmd

    hidden_state = np.ascontiguousarray(np.asarray(hidden_state, dtype=np.float32))
    W = np.asarray(W, dtype=np.float32)
    b = np.asarray(b, dtype=np.float32)
    group = np.asarray(group)
    labels = np.asarray(labels)

    if USE_FP32R:
        np_x = np_w = np_io = np.float32
        wscale = 1.0
    elif USE_FP8:
        np_x = np_w = ml_dtypes.float8_e4m3
        np_io = ml_dtypes.bfloat16
        wscale = W_SCALE
    else:
        np_x = np_w = np_io = ml_dtypes.bfloat16
        wscale = 1.0

    g64 = group.astype(np.int64)
    active = np.nonzero(g64 < NH)[0]
    order = np.argsort(g64[active], kind="stable")
    sidx = active[order]
    counts = np.bincount(g64[active], minlength=NH)

    # per-shard rows per group, padded to a multiple of 128
    n_seg = []
    for g in range(NH):
        n = math.ceil(counts[g] / PB) if counts[g] else 0
        n_seg.append(128 * math.ceil(n / 128) if n else 0)
    R = sum(n_seg)
    T = R // 128

    # deal rows: shard s takes every PB-th row of each group's sorted run
    idx = np.full((PB, R), -1, dtype=np.int64)
    off = 0
    roff = 0
    for g in range(NH):
        rows = sidx[off:off + counts[g]]
        for s in range(PB):
            sub = rows[s::PB]
            idx[s, roff:roff + len(sub)] = sub
        off += counts[g]
        roff += n_seg[g]

    # pack x per shard: [128, T*KT*128], M-tile-major so each tile is one
    # contiguous DMA: xp[p, (t*KT+h)*128 + r] = xg[t*128+r, h*128+p]
    xpacks = []
    for s in range(PB):
        xg = hidden_state[np.maximum(idx[s], 0)].astype(np_x)   # [R, H]
        xp = xg.reshape(T, 128, KT, 128).transpose(3, 0, 2, 1)  # [p, t, h, r]
        xpacks.append(np.ascontiguousarray(xp.reshape(128, T * KT * 128)))

    # pack W per L-half: [128, NH*KT*LS]; bias [1, NH*LS]
    wpacks = []
    bpacks = []
    for l in range(PL):
        parts = []
        for g in range(NH):
            wg = (W[g].T[:, l * LS:(l + 1) * LS] * wscale).astype(np_w)  # [H, LS]
            wg = wg.reshape(KT, 128, LS).transpose(1, 0, 2)     # [128, KT, LS]
            parts.append(wg.reshape(128, KT * LS))
        wpacks.append(np.ascontiguousarray(np.concatenate(parts, axis=1)))
        bpacks.append(
            np.ascontiguousarray(
                (b[:, l * LS:(l + 1) * LS] * wscale).astype(np_w).reshape(1, NH * LS)
            )
        )

    in_maps = []
    for c in range(N_CORES):
        s, l = divmod(c, PL)
        in_maps.append({"xp": xpacks[s], "wp": wpacks[l], "bp": bpacks[l]})

    nc = _build_program(n_seg)
    res = run_bass_kernel_spmd(nc, in_maps, list(range(N_CORES)))
    LAST_RESULTS = res

    out = np.empty((B, L), dtype=np.float32)
    lab_rows = g64 == NH
    out[lab_rows] = labels[lab_rows, None].astype(np.float32)
    for c in range(N_CORES):
        s, l = divmod(c, PL)
        yp = res.results[c]["y"].astype(np.float32)       # [128, T*LS]
        yg = yp.reshape(128, T, LS).transpose(1, 0, 2).reshape(R, LS)
        m = idx[s] >= 0
        out[idx[s][m], l * LS:(l + 1) * LS] = yg[m]
    return out
